# revision 19
# baseline (speedup 1.0000x reference)
"""Multi-head self-attention (8 heads, head_dim 64, n=4096, dim=256) on 8
Trainium2 NeuronCores.

Sharding: one attention head per core (tensor parallel on the heads axis of
to_qkv / to_out). Each core:
  A) computes the dual-layout projections qk = [q; k] and kq = [k; q]
     (each 128 x 4096 bf16, q/k on opposite partition halves) plus
     v^T (4096 x 64 bf16, augmented with a ones column so the softmax
     denominator falls out of the PE matmul),
  B) streams the 4096x4096 attention for its head: sim = k^T q in [j, i]
     layout, with PAIRS of j-tiles row-packed into the two 64-row halves of
     the PE array (weight loads alternate row groups, so they pull ahead and
     the two matmuls overlap), exp on the scalar engine (no max
     subtraction -- logits are O(8), far from fp32 overflow), and the
     attention-weighted value sum accumulated over j in PSUM.  i-block 0 is
     interleaved with stage A so the scalar engine starts early.
  C) normalizes by the fused row sum (copy-first + fast reciprocal so PSUM
     frees immediately), then an AllToAll gives every core the full
     512-channel hidden state for its own 512-token slice; the final
     projection + bias runs per core on that slice.
The host only reshapes/slices/casts inputs per core and concatenates the 8
disjoint token slices of the output.
"""

import os
import sys
from contextlib import ExitStack

for _p in ("/opt/trn_rl_repo",):
    if os.path.isdir(_p) and _p not in sys.path:
        sys.path.append(_p)

import ml_dtypes
import numpy as np

import concourse.bass as bass
import concourse.mybir as mybir
import concourse.tile as tile
from concourse import bacc
from concourse.bass_utils import run_bass_kernel_spmd

HEADS = 8
HD = 64           # head dim
DIM = 256         # model dim
N = 4096          # tokens (64*64)
HID = HEADS * HD  # 512
NB = 8            # token blocks
BLK = N // NB     # 512
NJ = N // 128     # 32 j-tiles of 128
NG = NJ // 2      # 16 sim groups (= row-packed pairs) per i-block
N_CORES = 8

F32 = mybir.dt.float32
BF16 = mybir.dt.bfloat16
EXP = mybir.ActivationFunctionType.Exp


def build_program():
    nc = bacc.Bacc("TRN2", target_bir_lowering=False, debug=False,
                   num_devices=N_CORES)
    x_d = nc.declare_dram_parameter("x", [DIM, N], BF16, isOutput=False)
    # columns: [wq|wk | wk|wq | wv] (wq pre-scaled by head_dim**-0.5)
    wqkvT_d = nc.declare_dram_parameter("wqkvT", [DIM, 320], BF16,
                                        isOutput=False)
    woT_d = nc.declare_dram_parameter("woT", [HID, DIM], BF16, isOutput=False)
    b_d = nc.declare_dram_parameter("bout", [DIM], F32, isOutput=False)
    y_d = nc.declare_dram_parameter("y", [DIM, BLK], F32, isOutput=True)

    with tile.TileContext(nc) as tc, ExitStack() as ctx:
        const = ctx.enter_context(tc.tile_pool(name="const", bufs=1))
        sbA = ctx.enter_context(tc.tile_pool(name="sbA", bufs=1))
        pexp = ctx.enter_context(tc.tile_pool(name="pexp", bufs=6))
        psml = ctx.enter_context(tc.tile_pool(name="psml", bufs=2))
        dram = ctx.enter_context(tc.tile_pool(name="dram", bufs=1,
                                              space="DRAM"))
        psO = ctx.enter_context(tc.tile_pool(name="psO", bufs=2,
                                             space="PSUM"))

        # ---- constants / persistent SBUF ----
        wqkvT_sb = const.tile([128, 2, 320], BF16)
        nc.sync.dma_start(wqkvT_sb[:],
                          wqkvT_d.rearrange("(c p) m -> p c m", p=128))


        x_sb = sbA.tile([128, 2, N], BF16)
        qk_sb = sbA.tile([128, N], BF16)   # partitions 0:64 = q, 64:128 = k
        kq_sb = sbA.tile([128, N], BF16)   # partitions 0:64 = k, 64:128 = q
        # v^T augmented with a ones column: [j, 0:64] = v^T, [j, 64] = 1
        vaug_sb = sbA.tile([128, NJ, 66], BF16)
        nc.vector.memset(vaug_sb[:, :, 64:65], 1.0)

        a2a_in = dram.tile([NB, HD, BLK], BF16)
        a2a_out = dram.tile([NB, HD, BLK], BF16)
        warm_in = dram.tile([128, 4], F32)
        warm_out = dram.tile([128, 4], F32)
        # tiny warm-up collective: absorbs CC init cost under the preamble
        nc.gpsimd.collective_compute(
            "AllReduce", mybir.AluOpType.add,
            replica_groups=[list(range(N_CORES))],
            ins=[warm_in.opt()], outs=[warm_out.opt()])

        pending = []  # [countdown, fn] emitted in order once countdown <= 0

        def schedule(fn, delay):
            pending.append([delay, fn])

        def tick():
            for it in pending:
                it[0] -= 1
            for it in [it for it in pending if it[0] <= 0]:
                pending.remove(it)
                it[1]()

        def drain():
            while pending:
                pending.pop(0)[1]()

        def emit_stage_a(b, psA):
            bs = slice(b * BLK, (b + 1) * BLK)
            for c in range(2):
                nc.sync.dma_start(x_sb[:, c, bs],
                                  x_d[c * 128:(c + 1) * 128, bs])
            ps_qk = psA.tile([128, BLK], F32, tag="pa", name=f"psqk_{b}")
            for c in range(2):
                nc.tensor.matmul(ps_qk[:], wqkvT_sb[:, c, 0:128],
                                 x_sb[:, c, bs],
                                 start=(c == 0), stop=(c == 1))
            nc.vector.tensor_copy(qk_sb[:, bs], ps_qk[:])
            ps_kq = psA.tile([128, BLK], F32, tag="pa", name=f"pskq_{b}")
            for c in range(2):
                nc.tensor.matmul(ps_kq[:], wqkvT_sb[:, c, 128:256],
                                 x_sb[:, c, bs],
                                 start=(c == 0), stop=(c == 1))
            nc.vector.tensor_copy(kq_sb[:, bs], ps_kq[:])
            ps_v = psA.tile([HD, BLK], F32, tag="pa", name=f"psv_{b}")
            for c in range(2):
                nc.tensor.matmul(ps_v[:], wqkvT_sb[:, c, 256:320],
                                 x_sb[:, c, bs],
                                 start=(c == 0), stop=(c == 1))
            v_sb = psml.tile([HD, BLK], BF16, tag="vsb", name=f"vsb_{b}")
            nc.vector.tensor_copy(v_sb[:], ps_v[:])
            # XBAR transpose into a contiguous scratch (strided dest is not
            # XBAR-safe), on the gpsimd queue so xbar-mode flips don't
            # interleave with the x loads on the sync queue.
            vt_sc = psml.tile([128, 4, HD], BF16, tag="vtsc",
                              name=f"vtsc_{b}")
            nc.scalar.dma_start_transpose(vt_sc[:], v_sb[:])
            nc.vector.tensor_copy(vaug_sb[:, 4 * b:4 * (b + 1), 0:64],
                                  vt_sc[:])

        def emit_sim_group(i, js, ps_out, pool, banks):
            # j-tiles in `js`, row-packed by parity: even j uses k from
            # kq_sb (array rows 0:64), odd j uses k from qk_sb (rows
            # 64:128), so consecutive matmuls alternate row groups.
            isl = slice(i * BLK, (i + 1) * BLK)
            gsz = len(js)
            psg = pool.tile([128, banks, BLK], F32, tag="psg",
                            name=f"psg_{i}_{js[0]}")
            for t, j in enumerate(js):
                if j % 2 == 0:
                    nc.tensor.matmul(psg[:, t, :],
                                     kq_sb[0:64, j * 128:(j + 1) * 128],
                                     qk_sb[0:64, isl],
                                     start=True, stop=True,
                                     tile_position=(0, 0))
                else:
                    nc.tensor.matmul(psg[:, t, :],
                                     qk_sb[64:128, j * 128:(j + 1) * 128],
                                     kq_sb[64:128, isl],
                                     start=True, stop=True,
                                     tile_position=(64, 0))
            pe = pexp.tile([128, 3, BLK], BF16, tag="pe",
                           name=f"pe_{i}_{js[0]}")
            nc.scalar.activation(pe[:, 0:gsz, :], psg[:, 0:gsz, :], EXP)

            def mk_outp():
                for t2, j2 in enumerate(js):
                    nc.tensor.matmul(ps_out[0:65, :],
                                     vaug_sb[:, j2, 0:65],
                                     pe[:, t2, :],
                                     start=(j2 == 0), stop=(j2 == NJ - 1))
            schedule(mk_outp, 2)
            tick()

        def emit_norm(i, ps_out):
            # part a (DVE only): copy out of PSUM so ps_out frees for the
            # next i-block, then the slow reciprocal -- scheduled behind the
            # final out' accumulation entries in the pending queue, shadowed
            # under the next i-block's compute.
            oall = psml.tile([128, BLK], F32, tag="oall", name=f"oall_{i}")
            r_sb = psml.tile([128, BLK], F32, tag="rsb", name=f"rsb_{i}")

            rrow = dram.tile([BLK], F32, tag="rrow", bufs=2,
                             name=f"rrow_{i}")

            def mk_norm_a():
                nc.vector.tensor_copy(oall[0:65, :], ps_out[0:65, :])
                nc.vector.reciprocal(r_sb[64:65, :], oall[64:65, :])
                nc.sync.dma_start(rrow[:], r_sb[64:65, :])
            schedule(mk_norm_a, 2)

            # part b (touches the PE): delayed a few groups into the next
            # i-block so the PE never waits on the reciprocal.
            def mk_norm_b():
                rrep_sb = psml.tile([HD, BLK], F32, tag="rrep",
                                    name=f"rrep_{i}")
                nc.sync.dma_start(
                    rrep_sb[:],
                    rrow.rearrange("(o n) -> o n", o=1).broadcast_to(
                        (HD, BLK)))
                outn = psml.tile([HD, BLK], BF16, tag="outn",
                                 name=f"outn_{i}")
                nc.vector.tensor_mul(outn[:], oall[0:HD, :], rrep_sb[:])
                nc.sync.dma_start(a2a_in[i], outn[:])
            schedule(mk_norm_b, 5)

        # ---- stage A interleaved with i-block 0 (pair-sized exp groups) --
        ps_out0 = psO.tile([128, BLK], F32, tag="psout", name="psout_0")
        with tc.tile_pool(name="psA", bufs=2, space="PSUM") as psA_pool, \
                tc.tile_pool(name="psB2", bufs=2, space="PSUM") as psB2:
            for b in range(NB):
                emit_stage_a(b, psA_pool)
                emit_sim_group(0, [4 * b, 4 * b + 1], ps_out0, psB2, 2)
                emit_sim_group(0, [4 * b + 2, 4 * b + 3], ps_out0, psB2, 2)
            emit_norm(0, ps_out0)

        # ---- i-blocks 1..7 (3-tile exp groups; PSUM pool swapped 4->6) ----
        G3 = [[2 * g, 2 * g + 1] for g in range(NJ // 2)]
        with tc.tile_pool(name="psB3", bufs=2, space="PSUM") as psB3:
            for i in range(1, NB):
                ps_out = psO.tile([128, BLK], F32, tag="psout",
                                  name=f"psout_{i}")
                for js in G3:
                    emit_sim_group(i, js, ps_out, psB3, 2)
                emit_norm(i, ps_out)
            drain()

            # ---- stage C: AllToAll over token blocks + output projection --
            nc.gpsimd.collective_compute(
                "AllToAll", mybir.AluOpType.bypass,
                replica_groups=[list(range(N_CORES))],
                ins=[a2a_in.opt()], outs=[a2a_out.opt()])

            woT_sb = const.tile([128, 4, DIM], BF16)
            nc.sync.dma_start(woT_sb[:],
                              woT_d.rearrange("(c p) m -> p c m", p=128))
            b_sb = const.tile([128, 2], F32)
            nc.sync.dma_start(b_sb[:], b_d.rearrange("(m p) -> p m", p=128))
            rhs_sb = sbA.tile([128, 4, BLK], BF16)
            a2a_r = a2a_out.rearrange("(c a) d t -> (a d) c t", c=4, a=2)
            ps_yt = psB3.tile([128, 3, BLK], F32, tag="psg", name="psy")
            for c in range(4):
                nc.gpsimd.dma_start(rhs_sb[:, c, :], a2a_r[:, c, :])
                for m in range(2):
                    nc.tensor.matmul(ps_yt[:, m, :],
                                     woT_sb[:, c, m * 128:(m + 1) * 128],
                                     rhs_sb[:, c, :],
                                     start=(c == 0), stop=(c == 3))
            for m in range(2):
                y_sb = psml.tile([128, BLK], F32, tag="ysb", name=f"ysb_{m}")
                nc.vector.tensor_scalar_add(y_sb[:], ps_yt[:, m, :],
                                            b_sb[:, m:m + 1])
                nc.sync.dma_start(y_d[m * 128:(m + 1) * 128, :], y_sb[:])

    nc.compile()
    return nc


def _make_in_maps(x, w_qkv, w_out, b_out):
    x2 = np.ascontiguousarray(
        np.asarray(x, np.float32).reshape(DIM, N)).astype(ml_dtypes.bfloat16)
    w_qkv = np.asarray(w_qkv, np.float32)
    scale = HD ** -0.5
    woT = np.ascontiguousarray(np.asarray(w_out, np.float32).T).astype(
        ml_dtypes.bfloat16)
    b = np.ascontiguousarray(np.asarray(b_out, np.float32).reshape(DIM))
    in_maps = []
    for h in range(N_CORES):
        wq = w_qkv[h * HD:(h + 1) * HD] * scale
        wk = w_qkv[HID + h * HD:HID + (h + 1) * HD]
        wv = w_qkv[2 * HID + h * HD:2 * HID + (h + 1) * HD]
        wqkvT = np.ascontiguousarray(
            np.concatenate([wq.T, wk.T, wk.T, wq.T, wv.T], axis=1),
            np.float32).astype(ml_dtypes.bfloat16)
        in_maps.append({"x": x2, "wqkvT": wqkvT, "woT": woT, "bout": b})
    return in_maps


def _assemble(results):
    y = np.concatenate([results[h]["y"] for h in range(N_CORES)], axis=1)
    return np.ascontiguousarray(y.reshape(1, DIM, 64, 64).astype(np.float32))


def kernel(x, w_qkv, w_out, b_out):
    nc = build_program()
    in_maps = _make_in_maps(x, w_qkv, w_out, b_out)
    res = run_bass_kernel_spmd(nc, in_maps, list(range(N_CORES)))
    return _assemble(res.results)


def run_traced(x, w_qkv, w_out, b_out, trace_cores=None):
    """Test-harness entry: also returns BassKernelResults with exec_time_ns."""
    nc = build_program()
    in_maps = _make_in_maps(x, w_qkv, w_out, b_out)
    res = run_bass_kernel_spmd(nc, in_maps, list(range(N_CORES)), trace=True,
                               trace_cores=trace_cores)
    return _assemble(res.results), res


# revision 20
# speedup vs baseline: 1.2939x; 1.2939x over previous
"""Multi-head self-attention (8 heads, head_dim 64, n=4096, dim=256) on 8
Trainium2 NeuronCores.

Sharding: one attention head per core (tensor parallel on the heads axis of
to_qkv / to_out). Each core:
  A) computes the dual-layout projections qk = [q; k] and kq = [k; q]
     (each 128 x 4096 bf16, q/k on opposite partition halves) plus
     v^T (4096 x 64 bf16, augmented with a ones column so the softmax
     denominator falls out of the PE matmul),
  B) streams the 4096x4096 attention for its head: sim = k^T q in [j, i]
     layout, with PAIRS of j-tiles row-packed into the two 64-row halves of
     the PE array (weight loads alternate row groups, so they pull ahead and
     the two matmuls overlap), exp on the scalar engine (no max
     subtraction -- logits are O(8), far from fp32 overflow), and the
     attention-weighted value sum accumulated over j in PSUM.  i-block 0 is
     interleaved with stage A so the scalar engine starts early.
  C) normalizes by the fused row sum (copy-first + fast reciprocal so PSUM
     frees immediately), then an AllToAll gives every core the full
     512-channel hidden state for its own 512-token slice; the final
     projection + bias runs per core on that slice.
The host only reshapes/slices/casts inputs per core and concatenates the 8
disjoint token slices of the output.
"""

import os
import sys
from contextlib import ExitStack

for _p in ("/opt/trn_rl_repo",):
    if os.path.isdir(_p) and _p not in sys.path:
        sys.path.append(_p)

import ml_dtypes
import numpy as np

import concourse.bass as bass
import concourse.mybir as mybir
import concourse.tile as tile
from concourse import bacc
from concourse.bass_utils import run_bass_kernel_spmd

HEADS = 8
HD = 64           # head dim
DIM = 256         # model dim
N = 4096          # tokens (64*64)
HID = HEADS * HD  # 512
NB = 8            # token blocks
BLK = N // NB     # 512
NJ = N // 128     # 32 j-tiles of 128
NG = NJ // 2      # 16 sim groups (= row-packed pairs) per i-block
N_CORES = 8

F32 = mybir.dt.float32
BF16 = mybir.dt.bfloat16
EXP = mybir.ActivationFunctionType.Exp


def build_program():
    nc = bacc.Bacc("TRN2", target_bir_lowering=False, debug=False,
                   num_devices=N_CORES)
    x_d = nc.declare_dram_parameter("x", [DIM, N], BF16, isOutput=False)
    # columns: [wq|wk | wk|wq | wv] (wq pre-scaled by head_dim**-0.5)
    wqkvT_d = nc.declare_dram_parameter("wqkvT", [DIM, 320], BF16,
                                        isOutput=False)
    woT_d = nc.declare_dram_parameter("woT", [HID, DIM], BF16, isOutput=False)
    b_d = nc.declare_dram_parameter("bout", [DIM], F32, isOutput=False)
    y_d = nc.declare_dram_parameter("y", [DIM, BLK], F32, isOutput=True)

    with tile.TileContext(nc) as tc, ExitStack() as ctx:
        const = ctx.enter_context(tc.tile_pool(name="const", bufs=1))
        sbA = ctx.enter_context(tc.tile_pool(name="sbA", bufs=1))
        pexp = ctx.enter_context(tc.tile_pool(name="pexp", bufs=6))
        psml = ctx.enter_context(tc.tile_pool(name="psml", bufs=2))
        dram = ctx.enter_context(tc.tile_pool(name="dram", bufs=1,
                                              space="DRAM"))
        psO = ctx.enter_context(tc.tile_pool(name="psO", bufs=2,
                                             space="PSUM"))

        # ---- constants / persistent SBUF ----
        wqkvT_sb = const.tile([128, 2, 320], BF16)
        nc.sync.dma_start(wqkvT_sb[:],
                          wqkvT_d.rearrange("(c p) m -> p c m", p=128))


        ones_sb = const.tile([128, HD], F32)
        nc.vector.memset(ones_sb[:], 1.0)
        x_sb = sbA.tile([128, 2, N], BF16)
        qk_sb = sbA.tile([128, N], BF16)   # partitions 0:64 = q, 64:128 = k
        kq_sb = sbA.tile([128, N], BF16)   # partitions 0:64 = k, 64:128 = q
        # v^T augmented with a ones column: [j, 0:64] = v^T, [j, 64] = 1
        vaug_sb = sbA.tile([128, NJ, 66], BF16)
        nc.vector.memset(vaug_sb[:, :, 64:65], 1.0)

        a2a_in = dram.tile([NB, HD, BLK], BF16)
        a2a_out = dram.tile([NB, HD, BLK], BF16)
        warm_in = dram.tile([128, 4], F32)
        warm_out = dram.tile([128, 4], F32)
        # tiny warm-up collective: absorbs CC init cost under the preamble
        nc.gpsimd.collective_compute(
            "AllReduce", mybir.AluOpType.add,
            replica_groups=[list(range(N_CORES))],
            ins=[warm_in.opt()], outs=[warm_out.opt()])

        pending = []  # [countdown, fn] emitted in order once countdown <= 0

        def schedule(fn, delay):
            pending.append([delay, fn])

        def tick():
            for it in pending:
                it[0] -= 1
            for it in [it for it in pending if it[0] <= 0]:
                pending.remove(it)
                it[1]()

        def drain():
            while pending:
                pending.pop(0)[1]()

        def emit_stage_a(b, psA):
            bs = slice(b * BLK, (b + 1) * BLK)
            for c in range(2):
                nc.sync.dma_start(x_sb[:, c, bs],
                                  x_d[c * 128:(c + 1) * 128, bs])
            ps_qk = psA.tile([128, BLK], F32, tag="pa", name=f"psqk_{b}")
            for c in range(2):
                nc.tensor.matmul(ps_qk[:], wqkvT_sb[:, c, 0:128],
                                 x_sb[:, c, bs],
                                 start=(c == 0), stop=(c == 1))
            nc.vector.tensor_copy(qk_sb[:, bs], ps_qk[:])
            ps_kq = psA.tile([128, BLK], F32, tag="pa", name=f"pskq_{b}")
            for c in range(2):
                nc.tensor.matmul(ps_kq[:], wqkvT_sb[:, c, 128:256],
                                 x_sb[:, c, bs],
                                 start=(c == 0), stop=(c == 1))
            nc.vector.tensor_copy(kq_sb[:, bs], ps_kq[:])
            for t in range(4):
                nt = b * 4 + t
                ps_v = psA.tile([128, HD], F32, tag="pa", name=f"psv_{nt}")
                for c in range(2):
                    nc.tensor.matmul(
                        ps_v[:],
                        x_sb[:, c, nt * 128:(nt + 1) * 128],
                        wqkvT_sb[:, c, 256:320],
                        start=(c == 0), stop=(c == 1))
                nc.vector.tensor_copy(vaug_sb[:, nt, 0:64], ps_v[:])

        def emit_sim_group(i, js, ps_out, pool, banks):
            # j-tiles in `js`, row-packed by parity: even j uses k from
            # kq_sb (array rows 0:64), odd j uses k from qk_sb (rows
            # 64:128), so consecutive matmuls alternate row groups.
            isl = slice(i * BLK, (i + 1) * BLK)
            gsz = len(js)
            psg = pool.tile([128, banks, BLK], F32, tag="psg",
                            name=f"psg_{i}_{js[0]}")
            for t, j in enumerate(js):
                if j % 2 == 0:
                    nc.tensor.matmul(psg[:, t, :],
                                     kq_sb[0:64, j * 128:(j + 1) * 128],
                                     qk_sb[0:64, isl],
                                     start=True, stop=True,
                                     tile_position=(0, 0))
                else:
                    nc.tensor.matmul(psg[:, t, :],
                                     qk_sb[64:128, j * 128:(j + 1) * 128],
                                     kq_sb[64:128, isl],
                                     start=True, stop=True,
                                     tile_position=(64, 0))
            pe = pexp.tile([128, 3, BLK], BF16, tag="pe",
                           name=f"pe_{i}_{js[0]}")
            nc.scalar.activation(pe[:, 0:gsz, :], psg[:, 0:gsz, :], EXP)

            def mk_outp():
                for t2, j2 in enumerate(js):
                    nc.tensor.matmul(ps_out[0:65, :],
                                     vaug_sb[:, j2, 0:65],
                                     pe[:, t2, :],
                                     start=(j2 == 0), stop=(j2 == NJ - 1))
            schedule(mk_outp, 2)
            tick()

        def emit_norm(i, ps_out):
            # part a (DVE only): copy out of PSUM so ps_out frees for the
            # next i-block, then the slow reciprocal -- scheduled behind the
            # final out' accumulation entries in the pending queue, shadowed
            # under the next i-block's compute.
            oall = psml.tile([128, BLK], F32, tag="oall", name=f"oall_{i}")
            r_sb = psml.tile([128, BLK], F32, tag="rsb", name=f"rsb_{i}")

            rrow = dram.tile([BLK], F32, tag="rrow", bufs=2,
                             name=f"rrow_{i}")

            def mk_norm_a():
                nc.vector.tensor_copy(oall[0:65, :], ps_out[0:65, :])
                nc.vector.reciprocal(r_sb[64:65, :], oall[64:65, :])
                nc.sync.dma_start(rrow[:], r_sb[64:65, :])
            schedule(mk_norm_a, 2)

            # part b (touches the PE): delayed a few groups into the next
            # i-block so the PE never waits on the reciprocal.
            def mk_norm_b():
                outn = psml.tile([HD, BLK], BF16, tag="outn",
                                 name=f"outn_{i}")
                if i == NB - 1:
                    # tail: PE is idle, replicate 1/s with a K=1 matmul
                    # (shorter dep chain than the DRAM-bounce broadcast)
                    ps_r = psO.tile([128, BLK], F32, tag="psout",
                                    name="psr_tail")
                    nc.tensor.matmul(ps_r[0:HD, :], ones_sb[64:65, 0:HD],
                                     r_sb[64:65, :], start=True, stop=True)
                    nc.vector.tensor_mul(outn[:], oall[0:HD, :],
                                         ps_r[0:HD, :])
                else:
                    rrep_sb = psml.tile([HD, BLK], F32, tag="rrep",
                                        name=f"rrep_{i}")
                    nc.sync.dma_start(
                        rrep_sb[:],
                        rrow.rearrange("(o n) -> o n", o=1).broadcast_to(
                            (HD, BLK)))
                    nc.vector.tensor_mul(outn[:], oall[0:HD, :], rrep_sb[:])
                nc.sync.dma_start(a2a_in[i], outn[:])
            schedule(mk_norm_b, 5)

        # ---- stage A interleaved with i-block 0 (pair-sized exp groups) --
        ps_out0 = psO.tile([128, BLK], F32, tag="psout", name="psout_0")
        with tc.tile_pool(name="psA", bufs=2, space="PSUM") as psA_pool, \
                tc.tile_pool(name="psB2", bufs=2, space="PSUM") as psB2:
            for b in range(NB):
                emit_stage_a(b, psA_pool)
                emit_sim_group(0, [4 * b, 4 * b + 1], ps_out0, psB2, 2)
                emit_sim_group(0, [4 * b + 2, 4 * b + 3], ps_out0, psB2, 2)
            emit_norm(0, ps_out0)

        # ---- i-blocks 1..7 (3-tile exp groups; PSUM pool swapped 4->6) ----
        G3 = [[2 * g, 2 * g + 1] for g in range(NJ // 2)]
        with tc.tile_pool(name="psB3", bufs=2, space="PSUM") as psB3:
            for i in range(1, NB):
                ps_out = psO.tile([128, BLK], F32, tag="psout",
                                  name=f"psout_{i}")
                for js in G3:
                    emit_sim_group(i, js, ps_out, psB3, 2)
                emit_norm(i, ps_out)
            drain()

            # ---- stage C: AllToAll over token blocks + output projection --
            nc.gpsimd.collective_compute(
                "AllToAll", mybir.AluOpType.bypass,
                replica_groups=[list(range(N_CORES))],
                ins=[a2a_in.opt()], outs=[a2a_out.opt()])

            woT_sb = const.tile([128, 4, DIM], BF16)
            nc.sync.dma_start(woT_sb[:],
                              woT_d.rearrange("(c p) m -> p c m", p=128))
            b_sb = const.tile([128, 2], F32)
            nc.sync.dma_start(b_sb[:], b_d.rearrange("(m p) -> p m", p=128))
            rhs_sb = sbA.tile([128, 4, BLK], BF16)
            a2a_r = a2a_out.rearrange("(c a) d t -> (a d) c t", c=4, a=2)
            ps_yt = psB3.tile([128, 3, BLK], F32, tag="psg", name="psy")
            for c in range(4):
                nc.gpsimd.dma_start(rhs_sb[:, c, :], a2a_r[:, c, :])
                for m in range(2):
                    nc.tensor.matmul(ps_yt[:, m, :],
                                     woT_sb[:, c, m * 128:(m + 1) * 128],
                                     rhs_sb[:, c, :],
                                     start=(c == 0), stop=(c == 3))
            for m in range(2):
                y_sb = psml.tile([128, BLK], F32, tag="ysb", name=f"ysb_{m}")
                nc.vector.tensor_scalar_add(y_sb[:], ps_yt[:, m, :],
                                            b_sb[:, m:m + 1])
                nc.sync.dma_start(y_d[m * 128:(m + 1) * 128, :], y_sb[:])

    nc.compile()
    return nc


def _make_in_maps(x, w_qkv, w_out, b_out):
    x2 = np.ascontiguousarray(
        np.asarray(x, np.float32).reshape(DIM, N)).astype(ml_dtypes.bfloat16)
    w_qkv = np.asarray(w_qkv, np.float32)
    scale = HD ** -0.5
    woT = np.ascontiguousarray(np.asarray(w_out, np.float32).T).astype(
        ml_dtypes.bfloat16)
    b = np.ascontiguousarray(np.asarray(b_out, np.float32).reshape(DIM))
    in_maps = []
    for h in range(N_CORES):
        wq = w_qkv[h * HD:(h + 1) * HD] * scale
        wk = w_qkv[HID + h * HD:HID + (h + 1) * HD]
        wv = w_qkv[2 * HID + h * HD:2 * HID + (h + 1) * HD]
        wqkvT = np.ascontiguousarray(
            np.concatenate([wq.T, wk.T, wk.T, wq.T, wv.T], axis=1),
            np.float32).astype(ml_dtypes.bfloat16)
        in_maps.append({"x": x2, "wqkvT": wqkvT, "woT": woT, "bout": b})
    return in_maps


def _assemble(results):
    y = np.concatenate([results[h]["y"] for h in range(N_CORES)], axis=1)
    return np.ascontiguousarray(y.reshape(1, DIM, 64, 64).astype(np.float32))


def kernel(x, w_qkv, w_out, b_out):
    nc = build_program()
    in_maps = _make_in_maps(x, w_qkv, w_out, b_out)
    res = run_bass_kernel_spmd(nc, in_maps, list(range(N_CORES)))
    return _assemble(res.results)


def run_traced(x, w_qkv, w_out, b_out, trace_cores=None):
    """Test-harness entry: also returns BassKernelResults with exec_time_ns."""
    nc = build_program()
    in_maps = _make_in_maps(x, w_qkv, w_out, b_out)
    res = run_bass_kernel_spmd(nc, in_maps, list(range(N_CORES)), trace=True,
                               trace_cores=trace_cores)
    return _assemble(res.results), res
